# revision 1
# baseline (speedup 1.0000x reference)
"""Trainium2 Bass kernel for nn_CrosslayerDecoder.

Reference computation:
    out[:, l, :] = sum_{i<=l} features[:, i, :] @ W_l[i]  + b[l]
with B=64, L=12, DF=4096, DA=768 (fp32).

The work is 78 independent [64,4096]@[4096,768] products (one per (l, i)
pair), each weight block read exactly once -> memory-bound on the ~981 MB
of weights.  Sharding: the 78 (l, i) pairs are split across the 8 cores
(10/10/10/10/10/10/9+pad/9+pad), so every per-core weight DMA is a large
fully-contiguous block.  Each core computes partial outputs [pairs, B, DA];
the host sums partials into layers and adds the bias.

Per pair on-device: featT block (pre-transposed on host to [128, 32*64])
is the stationary matmul operand; the weight block [4096, 768] streams
through the PE in 4 chunks of 8 k-tiles, accumulating in PSUM over the 32
k-tiles (float32r matmuls, full PE rate at free-dim 384).
"""

import numpy as np

import concourse.bass as bass  # noqa: F401  (bass types used via tile/bacc)
import concourse.mybir as mybir
import concourse.tile as tile
from concourse import bacc
from concourse.bass_utils import run_bass_kernel_spmd

B, L, DF, DA = 64, 12, 4096, 768
NCORES = 8
PAIRS_PER_CORE = 10
P = 128                  # SBUF partitions
KT = DF // P             # 32 k-tiles per pair
CH = 4                   # weight chunks per pair
KS = KT // CH            # 8 k-tiles per chunk
NH = DA // 2             # 384, matmul moving free dim (<=512 fp32)

# matmul input dtype: float32r streams at full PE rate for free-dim >= 256
# (plain float32 is quarter-rate).  Flip to mybir.dt.float32 if fp32r HW
# numerics turn out unacceptable.
MM_DT = mybir.dt.float32r

# i-major pair order: all (l, i) with l >= i, i ascending.  Consecutive
# runs go to consecutive cores; cores 6,7 have 9 real pairs + 1 zero pad.
_PAIRS = [(l, i) for i in range(L) for l in range(i, L)]
_COUNTS = [10, 10, 10, 10, 10, 10, 9, 9]
_ASSIGN: list[list[tuple[int, int] | None]] = []
_off = 0
for _c in _COUNTS:
    sl: list[tuple[int, int] | None] = list(_PAIRS[_off : _off + _c])
    sl += [None] * (PAIRS_PER_CORE - _c)
    _ASSIGN.append(sl)
    _off += _c
assert _off == len(_PAIRS) == 78

_NC_CACHE = None


def _build_program():
    """One SPMD Bass program (identical on all 8 cores)."""
    global _NC_CACHE
    if _NC_CACHE is not None:
        return _NC_CACHE

    nc = bacc.Bacc("TRN2", target_bir_lowering=False, debug=False)
    f_in = nc.dram_tensor(
        "fstack", [PAIRS_PER_CORE, P, KT * B], MM_DT, kind="ExternalInput"
    ).ap()
    w_in = nc.dram_tensor(
        "wstack", [PAIRS_PER_CORE, DF, DA], MM_DT, kind="ExternalInput"
    ).ap()
    o_out = nc.dram_tensor(
        "out", [PAIRS_PER_CORE, B, DA], mybir.dt.float32, kind="ExternalOutput"
    ).ap()

    with tile.TileContext(nc) as tc:
        with (
            tc.tile_pool(name="f", bufs=2) as fpool,
            tc.tile_pool(name="w", bufs=3) as wpool,
            tc.tile_pool(name="ps", bufs=2, space="PSUM") as pspool,
            tc.tile_pool(name="o", bufs=2) as opool,
        ):
            for p in range(PAIRS_PER_CORE):
                ft = fpool.tile([P, KT * B], MM_DT)
                nc.sync.dma_start(out=ft[:], in_=f_in[p])
                ps_a = pspool.tile([B, NH], mybir.dt.float32)
                ps_b = pspool.tile([B, NH], mybir.dt.float32)
                # [DF, DA] -> [chunk, partition, ktile-in-chunk, DA]
                w_rp = w_in[p].rearrange("(c ks pp) n -> c pp ks n", c=CH, ks=KS, pp=P)
                for c in range(CH):
                    wt = wpool.tile([P, KS * DA], MM_DT)
                    nc.sync.dma_start(
                        out=wt[:].rearrange("pp (ks n) -> pp ks n", ks=KS),
                        in_=w_rp[c],
                    )
                    for s in range(KS):
                        k = c * KS + s
                        lhs = ft[:, k * B : (k + 1) * B]
                        nc.tensor.matmul(
                            ps_a[:],
                            lhsT=lhs,
                            rhs=wt[:, s * DA : s * DA + NH],
                            start=(k == 0),
                            stop=(k == KT - 1),
                        )
                        nc.tensor.matmul(
                            ps_b[:],
                            lhsT=lhs,
                            rhs=wt[:, s * DA + NH : (s + 1) * DA],
                            start=(k == 0),
                            stop=(k == KT - 1),
                        )
                ot = opool.tile([B, DA], mybir.dt.float32)
                nc.vector.tensor_copy(ot[:, :NH], ps_a[:])
                nc.vector.tensor_copy(ot[:, NH:], ps_b[:])
                nc.sync.dma_start(out=o_out[p], in_=ot[:])

    nc.compile()
    _NC_CACHE = nc
    return nc


def _prep_inputs(features, Ws):
    """Per-core in_maps: pre-tiled feature blocks + stacked weight blocks."""
    features = np.ascontiguousarray(np.asarray(features, dtype=np.float32))
    # featT tile for feature index i: [DF, B] -> [P, KT*B] with layout
    # [partition, (ktile, batch)] so the SBUF tile is one contiguous DMA.
    ftiles = {}
    for i in range(L):
        x = features[:, i, :]                       # [B, DF]
        t = x.T.reshape(KT, P, B).transpose(1, 0, 2)  # [P, KT, B]
        ftiles[i] = np.ascontiguousarray(t.reshape(P, KT * B))

    in_maps = []
    for core in range(NCORES):
        fstack = np.zeros((PAIRS_PER_CORE, P, KT * B), dtype=np.float32)
        wstack = np.zeros((PAIRS_PER_CORE, DF, DA), dtype=np.float32)
        for slot, pair in enumerate(_ASSIGN[core]):
            if pair is None:
                continue
            l, i = pair
            fstack[slot] = ftiles[i]
            wstack[slot] = Ws[l][i]
        in_maps.append({"fstack": fstack, "wstack": wstack})
    return in_maps


def _assemble(results, b):
    out = np.zeros((B, L, DA), dtype=np.float32)
    for core in range(NCORES):
        o = np.asarray(results[core]["out"], dtype=np.float32)
        for slot, pair in enumerate(_ASSIGN[core]):
            if pair is None:
                continue
            l, _i = pair
            out[:, l, :] += o[slot]
    out += np.asarray(b, dtype=np.float32)[None, :, :]
    return out


def run(inputs: dict, trace: bool = False, **spmd_kwargs):
    """Compile (cached), run on 8 cores, return (full_output, BassKernelResults)."""
    Ws = [np.asarray(inputs[f"W_{l}"], dtype=np.float32) for l in range(L)]
    in_maps = _prep_inputs(inputs["features"], Ws)
    nc = _build_program()
    res = run_bass_kernel_spmd(
        nc, in_maps, list(range(NCORES)), trace=trace, **spmd_kwargs
    )
    out = _assemble(res.results, inputs["b"])
    return out, res


def kernel(**inputs) -> np.ndarray:
    out, _ = run(inputs)
    return out


# revision 2
# speedup vs baseline: 1.0440x; 1.0440x over previous
"""Trainium2 Bass kernel for nn_CrosslayerDecoder.

Reference computation:
    out[:, l, :] = sum_{i<=l} features[:, i, :] @ W_l[i]  + b[l]
with B=64, L=12, DF=4096, DA=768 (fp32).

The work is 78 independent [64,4096]@[4096,768] products (one per (l, i)
pair), each weight block read exactly once -> memory-bound on the ~981 MB
of weights.  Sharding: the 78 (l, i) pairs are split across the 8 cores
(10/10/10/10/10/10/9+pad/9+pad), so every per-core weight DMA is a large
fully-contiguous block.  Each core computes partial outputs [pairs, B, DA];
the host sums partials into layers and adds the bias.

Numerics: fp32 inputs are split on the host into bf16 hi + bf16 lo halves
(same total bytes over the wire).  Each k-tile contributes three bf16
matmuls (hi*hi, hi*lo, lo*hi) accumulated into the same fp32 PSUM group,
giving ~1e-5 relative error at full PE rate (plain fp32 matmul is
quarter-rate on the PE; float32r is full-rate but only tf32 precision).

All DRAM operands are pre-packed on the host into the exact SBUF tile
layout, so every DMA is a single fully-contiguous block transfer.
"""

import numpy as np
import ml_dtypes

import concourse.bass as bass  # noqa: F401
import concourse.mybir as mybir
import concourse.tile as tile
from concourse import bacc
from concourse.bass_utils import run_bass_kernel_spmd

B, L, DF, DA = 64, 12, 4096, 768
NCORES = 8
PAIRS_PER_CORE = 10
P = 128                  # SBUF partitions
KT = DF // P             # 32 k-tiles per pair
CH = 4                   # weight chunks per pair (per hi/lo stream)
KS = KT // CH            # 8 k-tiles per chunk
NH = DA // 2             # 384 = PSUM tile free dim (<=512 fp32 per bank)

BF16 = ml_dtypes.bfloat16

# i-major pair order: all (l, i) with l >= i, i ascending.  Consecutive
# runs go to consecutive cores; cores 6,7 have 9 real pairs + 1 zero pad.
_PAIRS = [(l, i) for i in range(L) for l in range(i, L)]
_COUNTS = [10, 10, 10, 10, 10, 10, 9, 9]
_ASSIGN: list[list[tuple[int, int] | None]] = []
_off = 0
for _c in _COUNTS:
    sl: list[tuple[int, int] | None] = list(_PAIRS[_off : _off + _c])
    sl += [None] * (PAIRS_PER_CORE - _c)
    _ASSIGN.append(sl)
    _off += _c
assert _off == len(_PAIRS) == 78

_NC_CACHE = None


def _build_program():
    """One SPMD Bass program (identical on all 8 cores)."""
    global _NC_CACHE
    if _NC_CACHE is not None:
        return _NC_CACHE

    dt = mybir.dt.bfloat16
    nc = bacc.Bacc("TRN2", target_bir_lowering=False, debug=False)
    fh_in = nc.dram_tensor(
        "f_hi", [PAIRS_PER_CORE, P, KT * B], dt, kind="ExternalInput"
    ).ap()
    fl_in = nc.dram_tensor(
        "f_lo", [PAIRS_PER_CORE, P, KT * B], dt, kind="ExternalInput"
    ).ap()
    wh_in = nc.dram_tensor(
        "w_hi", [PAIRS_PER_CORE, CH, P, KS * DA], dt, kind="ExternalInput"
    ).ap()
    wl_in = nc.dram_tensor(
        "w_lo", [PAIRS_PER_CORE, CH, P, KS * DA], dt, kind="ExternalInput"
    ).ap()
    o_out = nc.dram_tensor(
        "out", [PAIRS_PER_CORE, B, DA], mybir.dt.float32, kind="ExternalOutput"
    ).ap()

    with tile.TileContext(nc) as tc:
        with (
            tc.tile_pool(name="f", bufs=2) as fpool,
            tc.tile_pool(name="w", bufs=3) as wpool,
            tc.tile_pool(name="ps", bufs=2, space="PSUM") as pspool,
            tc.tile_pool(name="o", bufs=2) as opool,
        ):
            for p in range(PAIRS_PER_CORE):
                fh = fpool.tile([P, KT * B], dt, tag="fh")
                fl = fpool.tile([P, KT * B], dt, tag="fl")
                nc.sync.dma_start(out=fh[:], in_=fh_in[p])
                nc.sync.dma_start(out=fl[:], in_=fl_in[p])
                ps_a = pspool.tile([B, NH], mybir.dt.float32)
                ps_b = pspool.tile([B, NH], mybir.dt.float32)
                for c in range(CH):
                    wh = wpool.tile([P, KS * DA], dt, tag="wh")
                    wl = wpool.tile([P, KS * DA], dt, tag="wl")
                    nc.sync.dma_start(out=wh[:], in_=wh_in[p, c])
                    nc.sync.dma_start(out=wl[:], in_=wl_in[p, c])
                    for s in range(KS):
                        k = c * KS + s
                        lh = fh[:, k * B : (k + 1) * B]
                        ll = fl[:, k * B : (k + 1) * B]
                        whA = wh[:, s * DA : s * DA + NH]
                        whB = wh[:, s * DA + NH : (s + 1) * DA]
                        wlA = wl[:, s * DA : s * DA + NH]
                        wlB = wl[:, s * DA + NH : (s + 1) * DA]
                        first = k == 0
                        last = k == KT - 1
                        # hi*hi, hi*lo share the stationary lhsT tile
                        nc.tensor.matmul(ps_a[:], lhsT=lh, rhs=whA, start=first, stop=False)
                        nc.tensor.matmul(ps_b[:], lhsT=lh, rhs=whB, start=first, stop=False)
                        nc.tensor.matmul(ps_a[:], lhsT=lh, rhs=wlA, start=False, stop=False)
                        nc.tensor.matmul(ps_b[:], lhsT=lh, rhs=wlB, start=False, stop=False)
                        nc.tensor.matmul(ps_a[:], lhsT=ll, rhs=whA, start=False, stop=last)
                        nc.tensor.matmul(ps_b[:], lhsT=ll, rhs=whB, start=False, stop=last)
                ot = opool.tile([B, DA], mybir.dt.float32)
                nc.vector.tensor_copy(ot[:, :NH], ps_a[:])
                nc.vector.tensor_copy(ot[:, NH:], ps_b[:])
                nc.sync.dma_start(out=o_out[p], in_=ot[:])

    nc.compile()
    _NC_CACHE = nc
    return nc


def _split_bf16(x32):
    hi = x32.astype(BF16)
    lo = (x32 - hi.astype(np.float32)).astype(BF16)
    return hi, lo


def _pack_w(w32):
    """[DF, DA] fp32 -> hi/lo bf16 in SBUF chunk layout [CH, P, KS*DA]."""
    hi, lo = _split_bf16(w32)

    def pack(x):
        return np.ascontiguousarray(
            x.reshape(CH, KS, P, DA).transpose(0, 2, 1, 3).reshape(CH, P, KS * DA)
        )

    return pack(hi), pack(lo)


def _prep_inputs(features, Ws):
    """Per-core in_maps: pre-tiled bf16 hi/lo feature + weight blocks."""
    features = np.ascontiguousarray(np.asarray(features, dtype=np.float32))
    # featT tile for feature index i: [DF, B] -> [P, KT*B] with layout
    # [partition, (ktile, batch)] so the SBUF tile is one contiguous DMA.
    fh_tiles, fl_tiles = {}, {}
    for i in range(L):
        x = features[:, i, :]                          # [B, DF]
        t = x.T.reshape(KT, P, B).transpose(1, 0, 2)   # [P, KT, B]
        hi, lo = _split_bf16(np.ascontiguousarray(t.reshape(P, KT * B)))
        fh_tiles[i], fl_tiles[i] = hi, lo

    in_maps = []
    for core in range(NCORES):
        fh = np.zeros((PAIRS_PER_CORE, P, KT * B), dtype=BF16)
        fl = np.zeros((PAIRS_PER_CORE, P, KT * B), dtype=BF16)
        wh = np.zeros((PAIRS_PER_CORE, CH, P, KS * DA), dtype=BF16)
        wl = np.zeros((PAIRS_PER_CORE, CH, P, KS * DA), dtype=BF16)
        for slot, pair in enumerate(_ASSIGN[core]):
            if pair is None:
                continue
            l, i = pair
            fh[slot], fl[slot] = fh_tiles[i], fl_tiles[i]
            wh[slot], wl[slot] = _pack_w(np.asarray(Ws[l][i], dtype=np.float32))
        in_maps.append({"f_hi": fh, "f_lo": fl, "w_hi": wh, "w_lo": wl})
    return in_maps


def _assemble(results, b):
    out = np.zeros((B, L, DA), dtype=np.float32)
    for core in range(NCORES):
        o = np.asarray(results[core]["out"], dtype=np.float32)
        for slot, pair in enumerate(_ASSIGN[core]):
            if pair is None:
                continue
            l, _i = pair
            out[:, l, :] += o[slot]
    out += np.asarray(b, dtype=np.float32)[None, :, :]
    return out


def run(inputs: dict, trace: bool = False, **spmd_kwargs):
    """Compile (cached), run on 8 cores, return (full_output, BassKernelResults)."""
    Ws = [np.asarray(inputs[f"W_{l}"], dtype=np.float32) for l in range(L)]
    in_maps = _prep_inputs(inputs["features"], Ws)
    nc = _build_program()
    res = run_bass_kernel_spmd(
        nc, in_maps, list(range(NCORES)), trace=trace, **spmd_kwargs
    )
    out = _assemble(res.results, inputs["b"])
    return out, res


def kernel(**inputs) -> np.ndarray:
    out, _ = run(inputs)
    return out
